# revision 7
# baseline (speedup 1.0000x reference)
"""Trainium2 Bass kernel for nn_PhysicsInformedLayer (power-flow constraint loss).

Self-contained: kernel(**inputs) -> (v_mag, v_ang, constraint_loss).

Math identity (per batch b, edge e = (i -> j), z = r + ix):
    p_ij + i q_ij = V_i * conj(V_j / z),   V = v_mag * e^{i v_ang}
    p_calc[n] + i q_calc[n] = V_n * conj(G_n),  G_n = sum_{e: from=n} V_to[e] / z_e
so the per-edge from-gather disappears; only a to-gather of (U, W) =
(v_mag cos v_ang, v_mag sin v_ang) is needed, plus a segment-sum over
from-sorted edges, plus per-node finishing math.

Sharding: edge e lives on core to[e]//6400 (so the gather table per core is
its local 6400-node chunk). Within a core, 8 groups by from//6400, each
from-sorted; segment sums via fused multiply-scan + boundary ap_gather; the
partial per-node sums are ReduceScatter'ed across the 8 cores; each core
finishes its own 6400-node chunk (mis^2 + voltage loss partials).
"""

import numpy as np

B, N, E = 8, 50000, 800000
NC, NG, CHUNK = 8, 8, 6400
SUB = 432                 # PSUM / matmul / scan sub-chunk
GC = 864                  # gather / DMA chunk (2 * SUB)
C_PAD = 13824             # padded edges per (core, group) = 16*GC = 32*SUB
NSUB = C_PAD // SUB       # 32
NGC = C_PAD // GC         # 16
ZSLOT = CHUNK             # zero slot in UW table
TABW = CHUNK + 4          # 6404 (zero-slot cols 6400..6403)
BLK = 800                 # tail block width (CHUNK/8)
V_MIN, V_MAX, W_PB = 0.95, 1.05, 10.0

_CACHE = {}


# --------------------------------------------------------------------------
# custom DVE ops (registered at import into concourse.dve_ops)
# --------------------------------------------------------------------------

def _register_dve_ops():
    import concourse.dve_ops as dops
    from concourse.dve_spec import (
        Spec, Src0, Src1, C0, C1, Zero, AluOp, Bin, relu, sq, scan, lower,
    )
    from concourse.dve_uop import DveOpSpec

    if "MULSCAN_PF" in dops.CUSTOM_DVE_SPECS:
        return

    def ref_mulscan(in0, in1, s0, s1, imm2):
        x = in0.astype(np.float32) * (in1.astype(np.float32) ** 2)
        return np.asarray(s0, np.float32) + np.cumsum(x, axis=-1, dtype=np.float32)

    def ref_sqdiff(in0, in1, s0, s1, imm2):
        d = (in0.astype(np.float32) - in1.astype(np.float32)) ** 2
        acc = np.asarray(s0, np.float32) + d.reshape(d.shape[0], -1).sum(
            axis=-1, keepdims=True).astype(np.float32)
        return d, acc

    def ref_vloss(in0, in1, s0, s1, imm2):
        x = in0.astype(np.float32)
        lo = np.maximum(np.asarray(s0, np.float32) - x, 0.0)
        hi = np.maximum(x - np.asarray(s1, np.float32), 0.0)
        d = lo * lo + hi * hi
        acc = d.reshape(d.shape[0], -1).sum(axis=-1, keepdims=True).astype(np.float32)
        return d, acc

    defs = [
        # cs = s0 + cumsum(in0 * in1^2): T = num * (1/sqrt(z2))^2, scan-fused
        ("MULSCAN_PF", Spec(body=scan(AluOp.ADD, Src0 * sq(Src1), init=C0),
                            reference=ref_mulscan), True),
        # out = (in0-in1)^2 ; accum_out = s0 + sum(out)
        ("SQDIFF_PF", Spec(body=sq(Src0 - Src1), accum=AluOp.ADD, accum_init=C0,
                           reference=ref_sqdiff), True),
        # out = relu(s0-x)^2 + relu(x-s1)^2 ; accum_out = sum(out)
        ("VLOSS_PF", Spec(body=sq(relu(C0 - Src0)) + sq(relu(Src0 - C1)),
                          accum=AluOp.ADD, accum_init=Zero,
                          reference=ref_vloss), False),
    ]
    for name, spec, has_src1 in defs:
        row = dops._CUSTOM_DVE_ROW_BASE + len(dops.OPS)
        assert row < 0x20
        shas = {}
        for ver in ("v3", "v4"):
            tmp = DveOpSpec(name=name, opcode=row, uops=lower(spec, ver=ver),
                            rd1_en=has_src1)
            shas[ver] = tmp.sha(ver)
        op = dops.DveOp(name, spec, subdim=False, uops_sha=shas)
        dops.OPS.append(op)
        dops.CUSTOM_DVE_SPECS[name] = spec
        dops._SUB_OPCODE_FOR_NAME[name] = row
    return


def _get_op(name):
    import concourse.dve_ops as dops
    return {op.name: op for op in dops.OPS}[name]


# --------------------------------------------------------------------------
# device kernel (traced once, SPMD across 8 cores)
# --------------------------------------------------------------------------

def _trace_kernel(tc, outs, ins):
    import concourse.tile as tile
    from concourse import bass, mybir

    nc = tc.nc
    f32, bf16, i16 = mybir.dt.float32, mybir.dt.bfloat16, mybir.dt.int16
    AF = mybir.ActivationFunctionType
    PI_2 = float(np.pi / 2)

    rx_d, rxs_d, gidx_d, bidx_d, vm16_d, ang16_d, inj_d, mats_d, b16_d = ins
    misq_d, vq_d = outs

    MULSCAN = _get_op("MULSCAN_PF")
    SQDIFF = _get_op("SQDIFF_PF")
    VLOSS = _get_op("VLOSS_PF")

    import contextlib
    ctx = contextlib.ExitStack()
    with ctx:
        sb = ctx.enter_context(tc.tile_pool(name="sb", bufs=1))
        sbc = ctx.enter_context(tc.tile_pool(name="sbc", bufs=2))
        ps = ctx.enter_context(tc.tile_pool(name="ps", bufs=2, space="PSUM"))
        dram = ctx.enter_context(tc.tile_pool(name="dram", bufs=1, space="DRAM"))

        # ---- persistent tiles
        tab = sb.tile([128, TABW], f32, tag="tab")
        cs = sb.tile([128, 1 + C_PAD], f32, tag="cs")
        mats = sb.tile([128, 384], bf16, tag="mats")
        gidx = sb.tile([128, C_PAD // 16], i16, tag="gidx")
        bidx = sb.tile([128, CHUNK // 16], i16, tag="bidx")

        nc.sync.dma_start(mats[:], mats_d[:])
        nc.sync.dma_start(gidx[:], gidx_d[:])
        nc.sync.dma_start(bidx[:], bidx_d[:])
        A_m = mats[:, 0:128]
        B_m = mats[:, 128:256]
        Gz_m = mats[:, 256:384]

        nc.vector.memset(cs[:, 0:1], 0.0)

        b16 = sb.tile([16, 1], f32, tag="b16")
        nc.sync.dma_start(b16[:], b16_d[:])

        # ---- build UW table: U = vm*cos(ang) rows 0-7, W = vm*sin rows 8-15
        BW = 1601  # 4 build chunks
        for c in range(4):
            sl = slice(BW * c, BW * (c + 1))
            ang_c = sb.tile([16, BW], f32, tag="ang")
            vm_c = sb.tile([16, BW], f32, tag="vmb")
            trig_c = sb.tile([16, BW], f32, tag="trig")
            uw_c = sb.tile([16, BW], f32, tag="uwb")
            nc.sync.dma_start(ang_c[:], ang16_d[:, sl])
            nc.sync.dma_start(vm_c[:], vm16_d[:, sl])
            nc.scalar.activation(trig_c[:], ang_c[:], AF.Sin, bias=b16[:])
            nc.vector.tensor_mul(uw_c[:], trig_c[:], vm_c[:])
            for g in range(NG):
                nc.sync.dma_start(tab[16 * g:16 * g + 16, sl], uw_c[:])

        # ---- tail coefficient tiles (from tab rows 0:16) + inj + vm64
        UW128 = sb.tile([128, BLK], f32, tag="uw128")
        UW128s = sb.tile([128, BLK], f32, tag="uw128s")
        inj = sb.tile([128, BLK], f32, tag="inj")
        vm64 = sb.tile([64, BLK], f32, tag="vm64")
        nc.sync.dma_start(inj[:], inj_d[:])
        for blk in range(8):
            sl = slice(BLK * blk, BLK * (blk + 1))
            rows = slice(16 * blk, 16 * blk + 16)
            nc.sync.dma_start(UW128[rows, :], tab[0:16, sl])
            nc.sync.dma_start(UW128s[16 * blk:16 * blk + 8, :], tab[8:16, sl])
            nc.sync.dma_start(UW128s[16 * blk + 8:16 * blk + 16, :], tab[0:8, sl])
            nc.sync.dma_start(vm64[8 * blk:8 * blk + 8, :], vm16_d[0:8, sl])

        # ---- main edge pipeline
        for gc in range(NGC):
            esl = slice(GC * gc, GC * (gc + 1))
            isl = slice((GC // 16) * gc, (GC // 16) * (gc + 1))
            rx_c = sbc.tile([128, GC], f32, tag="rx")
            rxs_c = sbc.tile([128, GC], bf16, tag="rxs")
            uwg_c = sbc.tile([128, GC], f32, tag="uwg")
            t_c = sbc.tile([128, GC], bf16, tag="t")
            s_c = sbc.tile([128, GC], bf16, tag="s")
            sq_c = sbc.tile([128, GC], bf16, tag="sq")
            nc.sync.dma_start(rx_c[:], rx_d[:, esl])
            nc.sync.dma_start(rxs_c[:], rxs_d[:, esl])
            nc.gpsimd.ap_gather(
                out_ap=uwg_c[:], in_ap=tab[:], idxs_ap=gidx[:, isl],
                channels=128, num_elems=TABW, d=1, num_idxs=GC)
            nc.vector.tensor_mul(t_c[:], uwg_c[:], rx_c[:])
            nc.vector.tensor_mul(s_c[:], uwg_c[:], rxs_c[:])
            nc.scalar.square(sq_c[:], rx_c[:])
            for h in range(GC // SUB):
                ssl = slice(SUB * h, SUB * (h + 1))
                col0 = GC * gc + SUB * h  # global sub-chunk start
                psT = ps.tile([128, SUB], f32, tag="psT")
                psZ = ps.tile([128, SUB], f32, tag="psZ")
                y_c = sbc.tile([128, SUB], f32, tag="y")
                nc.tensor.matmul(out=psT[:], lhsT=A_m, rhs=t_c[:, ssl],
                                 start=True, stop=False)
                nc.tensor.matmul(out=psT[:], lhsT=B_m, rhs=s_c[:, ssl],
                                 start=False, stop=True)
                nc.tensor.matmul(out=psZ[:], lhsT=Gz_m, rhs=sq_c[:, ssl],
                                 start=True, stop=True)
                nc.scalar.activation(y_c[:], psZ[:], AF.Abs_reciprocal_sqrt)
                nc.vector._custom_dve(
                    MULSCAN, out=cs[:, 1 + col0:1 + col0 + SUB],
                    in0=psT[:], in1=y_c[:], s0=cs[:, col0:col0 + 1])

        # ---- boundary gather + diff -> per-core partial G [128, 6400]
        bndG = sb.tile([128, 1 + CHUNK], f32, tag="bndG")
        G = sb.tile([128, CHUNK], f32, tag="tab")  # reuse tab's slot (dead)
        nc.vector.memset(bndG[:, 0:1], 0.0)
        nc.gpsimd.ap_gather(
            out_ap=bndG[:, 1:1 + CHUNK], in_ap=cs[:], idxs_ap=bidx[:],
            channels=128, num_elems=1 + C_PAD, d=1, num_idxs=CHUNK)
        nc.vector.tensor_sub(G[:], bndG[:, 1:1 + CHUNK], bndG[:, 0:CHUNK])

        # ---- cross-core ReduceScatter (rows 16k..16k+16 -> core k)
        cc_in = dram.tile([128, CHUNK], f32)
        cc_out = dram.tile([16, CHUNK], f32)
        nc.gpsimd.dma_start(cc_in[:], G[:])
        nc.gpsimd.collective_compute(
            "ReduceScatter", mybir.AluOpType.add,
            replica_groups=[list(range(NC))],
            ins=[cc_in.opt()], outs=[cc_out.opt()])

        # ---- tail: this core's 6400 nodes reshaped [16,6400] -> [128,800]
        G128 = sb.tile([128, BLK], f32, tag="g128")
        for blk in range(8):
            nc.sync.dma_start(G128[16 * blk:16 * blk + 16, :],
                              cc_out[:, BLK * blk:BLK * (blk + 1)])
        t128 = sb.tile([128, BLK], bf16, tag="t128")
        s128 = sb.tile([128, BLK], bf16, tag="s128")
        nc.vector.tensor_mul(t128[:], UW128[:], G128[:])
        nc.vector.tensor_mul(s128[:], UW128s[:], G128[:])

        acc0 = sb.tile([128, 1], f32, tag="acc0")
        acc1 = sb.tile([128, 1], f32, tag="acc1")
        acc2 = sb.tile([128, 1], f32, tag="acc2")
        mscr = sb.tile([128, 400], bf16, tag="mscr")
        nc.vector.memset(acc0[:], 0.0)
        accs = [acc0, acc1, acc2]
        for h in range(2):
            ssl = slice(400 * h, 400 * (h + 1))
            psM = ps.tile([128, 400], f32, tag="psM")
            nc.tensor.matmul(out=psM[:], lhsT=A_m, rhs=t128[:, ssl],
                             start=True, stop=False)
            nc.tensor.matmul(out=psM[:], lhsT=B_m, rhs=s128[:, ssl],
                             start=False, stop=True)
            nc.vector._custom_dve(
                SQDIFF, out=mscr[:], in0=psM[:], in1=inj[:, ssl],
                s0=accs[h][:], accum_out=accs[h + 1][:])

        vq_t = sb.tile([64, 1], f32, tag="vqt")
        vscr = sb.tile([64, BLK], bf16, tag="vscr")
        nc.vector._custom_dve(
            VLOSS, out=vscr[:], in0=vm64[:], s0=V_MIN, s1=V_MAX,
            accum_out=vq_t[:])

        nc.sync.dma_start(misq_d[:], acc2[:])
        nc.sync.dma_start(vq_d[:], vq_t[:])


# --------------------------------------------------------------------------
# module build (cached)
# --------------------------------------------------------------------------

def _build_module():
    if "nc" in _CACHE:
        return _CACHE["nc"], _CACHE["io"]
    _register_dve_ops()
    import concourse.bacc as bacc
    import concourse.tile as tile
    from concourse import mybir

    nc = bacc.Bacc("TRN2", target_bir_lowering=False, debug=False,
                   num_devices=NC)
    f32, bf16, i16 = mybir.dt.float32, mybir.dt.bfloat16, mybir.dt.int16
    ins = [
        nc.dram_tensor("rx", [128, C_PAD], f32, kind="ExternalInput").ap(),
        nc.dram_tensor("rxs", [128, C_PAD], bf16, kind="ExternalInput").ap(),
        nc.dram_tensor("gidx", [128, C_PAD // 16], i16, kind="ExternalInput").ap(),
        nc.dram_tensor("bidx", [128, CHUNK // 16], i16, kind="ExternalInput").ap(),
        nc.dram_tensor("vm16", [16, TABW], f32, kind="ExternalInput").ap(),
        nc.dram_tensor("ang16", [16, TABW], f32, kind="ExternalInput").ap(),
        nc.dram_tensor("inj", [128, BLK], f32, kind="ExternalInput").ap(),
        nc.dram_tensor("mats", [128, 384], bf16, kind="ExternalInput").ap(),
        nc.dram_tensor("b16", [16, 1], f32, kind="ExternalInput").ap(),
    ]
    outs = [
        nc.dram_tensor("misq", [128, 1], f32, kind="ExternalOutput").ap(),
        nc.dram_tensor("vq", [64, 1], f32, kind="ExternalOutput").ap(),
    ]
    with tile.TileContext(nc) as tc:
        _trace_kernel(tc, outs, ins)
    nc.compile()
    _CACHE["nc"] = nc
    _CACHE["io"] = ([t.tensor.name for t in ins], [t.tensor.name for t in outs])
    return nc, _CACHE["io"]


# --------------------------------------------------------------------------
# host-side prep / unshard
# --------------------------------------------------------------------------

def _pair_matrices():
    A = np.zeros((128, 128), np.float32)
    Bm = np.zeros((128, 128), np.float32)
    Gz = np.zeros((128, 128), np.float32)
    for m in range(128):
        g, r = divmod(m, 16)
        if r < 8:  # u-row: re = t_u + t_w
            A[m, m] = 1.0
            A[m + 8, m] = 1.0
        else:      # w-row: im = s_u - s_w
            Bm[m - 8, m] = 1.0
            Bm[m, m] = -1.0
        mb = r % 8
        Gz[16 * g + mb, m] = 1.0
        Gz[16 * g + 8 + mb, m] = 1.0
    return A, Bm, Gz


def _host_prep(v_mag, v_ang, r_line, x_line, p_inj, q_inj, edge_index):
    import ml_dtypes
    frm = np.ascontiguousarray(edge_index[0]).astype(np.int64)
    to = np.ascontiguousarray(edge_index[1]).astype(np.int64)
    core_of = to // CHUNK
    grp_of = frm // CHUNK
    cell = core_of * NG + grp_of
    order = np.lexsort((frm, cell))
    cell_s = cell[order]
    frm_s = frm[order]
    to_s = to[order]
    counts = np.bincount(cell_s, minlength=NC * NG)
    assert counts.max() <= C_PAD, counts.max()
    starts = np.zeros(NC * NG + 1, np.int64)
    np.cumsum(counts, out=starts[1:])
    rank = np.arange(E, dtype=np.int64) - starts[cell_s]

    # padded per-(core,group) slot arrays
    gidx_all = np.full((NC, NG, C_PAD), ZSLOT, np.int32)
    rx_all = np.ones((NC, 128, C_PAD), np.float32)
    rxs_all = np.ones((NC, 128, C_PAD), np.float32)
    kk, gg = core_of[order], grp_of[order]
    gidx_all[kk, gg, rank] = (to_s - CHUNK * kk).astype(np.int32)
    eidx = order  # original edge ids in sorted order
    for b in range(B):
        rb = r_line[b]
        xb = x_line[b]
        rx_all[kk, 16 * gg + b, rank] = rb[eidx]
        rx_all[kk, 16 * gg + 8 + b, rank] = xb[eidx]
        rxs_all[kk, 16 * gg + b, rank] = xb[eidx]
        rxs_all[kk, 16 * gg + 8 + b, rank] = rb[eidx]

    A, Bm, Gz = _pair_matrices()
    bias16 = np.array([[np.pi / 2]] * 8 + [[0.0]] * 8, np.float32)
    mats = np.concatenate([A, Bm, Gz], axis=1).astype(ml_dtypes.bfloat16)

    in_maps = []
    for k in range(NC):
        # wrapped gather idx: [16g+p, 96*gc+s] = idx[g, gc*1536 + s*16 + p]
        gi = gidx_all[k].reshape(NG, NGC, GC // 16, 16)
        gi = gi.transpose(0, 3, 1, 2).reshape(NG * 16, NGC * (GC // 16))
        # interleave to [128, ...]: rows 16g+p
        gidx_w = gi.reshape(NG, 16, -1).reshape(128, -1).astype(np.int16)

        # boundary positions
        bidx_w = np.zeros((128, CHUNK // 16), np.int16)
        lo = CHUNK * k
        hi = min(CHUNK * (k + 1), N)
        n_real = hi - lo
        for g in range(NG):
            c0, c1 = starts[k * NG + g], starts[k * NG + g + 1]
            pos = np.searchsorted(frm_s[c0:c1], CHUNK * g + np.arange(CHUNK),
                                  side="right").astype(np.int16)
            bidx_w[16 * g:16 * g + 16] = pos.reshape(CHUNK // 16, 16).T

        vm16 = np.zeros((16, TABW), np.float32)
        ang16 = np.zeros((16, TABW), np.float32)
        for uw in range(2):
            vm16[uw * 8:uw * 8 + 8, :n_real] = v_mag[:, lo:hi]
            vm16[uw * 8:uw * 8 + 8, n_real:CHUNK] = 1.0
            ang16[uw * 8:uw * 8 + 8, :n_real] = v_ang[:, lo:hi]

        inj = np.zeros((128, BLK), np.float32)
        for blk in range(8):
            nlo = lo + BLK * blk
            w = max(0, min(nlo + BLK, hi) - nlo)
            if w > 0:
                inj[blk * 16:blk * 16 + 8, :w] = p_inj[:, nlo:nlo + w]
                inj[blk * 16 + 8:blk * 16 + 16, :w] = q_inj[:, nlo:nlo + w]

        in_maps.append({
            "rx": rx_all[k],
            "rxs": rxs_all[k].astype(ml_dtypes.bfloat16),
            "gidx": gidx_w,
            "bidx": bidx_w,
            "vm16": vm16,
            "ang16": ang16,
            "inj": inj,
            "mats": mats,
            "b16": bias16,
        })
    return in_maps


def kernel(**inputs):
    inputs = {k: np.asarray(v) for k, v in inputs.items()}
    nc, _ = _build_module()
    in_maps = _host_prep(**inputs)
    from concourse.bass_utils import run_bass_kernel_spmd
    res = run_bass_kernel_spmd(nc, in_maps, core_ids=list(range(NC)))
    mis_total = 0.0
    v_total = 0.0
    for k in range(NC):
        mis_total += float(res.results[k]["misq"].sum(dtype=np.float64))
        v_total += float(res.results[k]["vq"].sum(dtype=np.float64))
    loss = np.float32(W_PB * (mis_total / B) + v_total / B)
    return inputs["v_mag"], inputs["v_ang"], loss


# revision 22
# speedup vs baseline: 99.1482x; 99.1482x over previous
"""Trainium2 Bass kernel for nn_PhysicsInformedLayer (power-flow constraint loss).

Self-contained: kernel(**inputs) -> (v_mag, v_ang, constraint_loss).

Math identity (per batch b, edge e = (i -> j), z = r + ix):
    p_ij + i q_ij = V_i * conj(V_j / z),   V = v_mag * e^{i v_ang}
    p_calc[n] + i q_calc[n] = V_n * conj(G_n),  G_n = sum_{e: from=n} V_to[e] / z_e
so the per-edge from-gather disappears; only a to-gather of (U, W) =
(v_mag cos v_ang, v_mag sin v_ang) is needed, plus a segment-sum over
from-sorted edges, plus per-node finishing math.

Sharding: edge e lives on core to[e]//6400 (so the gather table per core is
its local 6400-node chunk). Within a core, 8 groups by from//6400, each
from-sorted; segment sums via fused multiply-scan + boundary ap_gather; the
partial per-node sums are ReduceScatter'ed across the 8 cores; each core
finishes its own 6400-node chunk (mis^2 + voltage loss partials).
"""

import numpy as np

B, N, E = 8, 50000, 800000
NC, NG, CHUNK = 8, 8, 6400
SUB = 512                 # PSUM / matmul / scan sub-chunk
GC = 1536                 # DMA/compute chunk (3 * SUB)
GAC = 4608                # ap_gather call width (3 * GC)
C_PAD = 13824             # padded edges per (core, group) = 9*GC = 27*SUB = 3*GAC
NSUB = C_PAD // SUB       # 27
NGC = C_PAD // GC         # 9
NGA = C_PAD // GAC        # 3
ZSLOT = CHUNK             # zero slot in UW table
TABW = CHUNK + 4          # 6404 (zero-slot cols 6400..6403)
BLK = 800                 # tail block width (CHUNK/8)
BND_M = 3200              # node split for the two boundary-gather calls
BND_T1W = 15 * SUB + 1    # 7681: call-1 table = cs[:, 0:BND_T1W]
BND_OFF2 = 6144           # call-2 table offset into cs
V_MIN, V_MAX, W_PB = 0.95, 1.05, 10.0

_CACHE = {}


# --------------------------------------------------------------------------
# custom DVE ops (registered at import into concourse.dve_ops)
# --------------------------------------------------------------------------

def _register_dve_ops():
    import concourse.dve_ops as dops
    from concourse.dve_spec import (
        Spec, Src0, Src1, C0, C1, Zero, AluOp, Bin, relu, sq, scan, lower,
    )
    from concourse.dve_uop import DveOpSpec

    if "MULSCAN_PF" in dops.CUSTOM_DVE_SPECS:
        return

    def ref_mulscan(in0, in1, s0, s1, imm2):
        x = in0.astype(np.float32) * (in1.astype(np.float32) ** 2)
        return np.asarray(s0, np.float32) + np.cumsum(x, axis=-1, dtype=np.float32)

    def ref_sqdiff(in0, in1, s0, s1, imm2):
        d = (in0.astype(np.float32) - in1.astype(np.float32)) ** 2
        acc = np.asarray(s0, np.float32) + d.reshape(d.shape[0], -1).sum(
            axis=-1, keepdims=True).astype(np.float32)
        return d, acc

    def ref_vloss(in0, in1, s0, s1, imm2):
        x = in0.astype(np.float32)
        lo = np.maximum(np.asarray(s0, np.float32) - x, 0.0)
        hi = np.maximum(x - np.asarray(s1, np.float32), 0.0)
        d = lo * lo + hi * hi
        acc = d.reshape(d.shape[0], -1).sum(axis=-1, keepdims=True).astype(np.float32)
        return d, acc

    defs = [
        # cs = s0 + cumsum(in0 * in1^2): T = num * (1/sqrt(z2))^2, scan-fused
        ("MULSCAN_PF", Spec(body=scan(AluOp.ADD, Src0 * sq(Src1), init=C0),
                            reference=ref_mulscan), True),
        # out = (in0-in1)^2 ; accum_out = s0 + sum(out)
        ("SQDIFF_PF", Spec(body=sq(Src0 - Src1), accum=AluOp.ADD, accum_init=C0,
                           reference=ref_sqdiff), True),
        # out = relu(s0-x)^2 + relu(x-s1)^2 ; accum_out = sum(out)
        ("VLOSS_PF", Spec(body=sq(relu(C0 - Src0)) + sq(relu(Src0 - C1)),
                          accum=AluOp.ADD, accum_init=Zero,
                          reference=ref_vloss), False),
    ]
    for name, spec, has_src1 in defs:
        row = dops._CUSTOM_DVE_ROW_BASE + len(dops.OPS)
        assert row < 0x20
        shas = {}
        for ver in ("v3", "v4"):
            tmp = DveOpSpec(name=name, opcode=row, uops=lower(spec, ver=ver),
                            rd1_en=has_src1)
            shas[ver] = tmp.sha(ver)
        op = dops.DveOp(name, spec, subdim=False, uops_sha=shas)
        dops.OPS.append(op)
        dops.CUSTOM_DVE_SPECS[name] = spec
        dops._SUB_OPCODE_FOR_NAME[name] = row
    return


def _get_op(name):
    import concourse.dve_ops as dops
    return {op.name: op for op in dops.OPS}[name]


# --------------------------------------------------------------------------
# device kernel (traced once, SPMD across 8 cores)
# --------------------------------------------------------------------------

def _trace_kernel(tc, outs, ins, single=False):
    import concourse.tile as tile
    from concourse import bass, mybir

    nc = tc.nc
    f32, bf16, i16 = mybir.dt.float32, mybir.dt.bfloat16, mybir.dt.int16
    AF = mybir.ActivationFunctionType
    PI_2 = float(np.pi / 2)

    rx_d, gidx_d, bidx_d, vm128_d, ang128_d, inj_d, mats_d, swp_d, b16_d = ins
    misq_d, vq_d = outs

    MULSCAN = _get_op("MULSCAN_PF")
    SQDIFF = _get_op("SQDIFF_PF")
    VLOSS = _get_op("VLOSS_PF")

    import contextlib
    ctx = contextlib.ExitStack()
    with ctx:
        sb = ctx.enter_context(tc.tile_pool(name="sb", bufs=1))
        sbc = ctx.enter_context(tc.tile_pool(name="sbc", bufs=2))
        ps = ctx.enter_context(tc.tile_pool(name="ps", bufs=2, space="PSUM"))
        sb1 = ctx.enter_context(tc.tile_pool(name="sb1", bufs=1))
        sbX = ctx.enter_context(tc.tile_pool(name="sbX", bufs=1))
        dram = ctx.enter_context(tc.tile_pool(name="dram", bufs=1, space="DRAM"))

        # ---- persistent tiles
        tab = sb.tile([128, TABW], f32, tag="tab")
        cs = sb.tile([128, 1 + C_PAD], f32, tag="cs")
        mats = sb.tile([128, 384], bf16, tag="mats")
        gidx = sb.tile([128, C_PAD // 16], i16, tag="gidx")
        bidx = sb.tile([128, CHUNK // 16], i16, tag="bidx")

        nc.sync.dma_start(mats[:], mats_d[:])
        nc.sync.dma_start(gidx[:], gidx_d[:])
        nc.sync.dma_start(bidx[:], bidx_d[:])
        A_m = mats[:, 0:128]
        B_m = mats[:, 128:256]
        Gz_m = mats[:, 256:384]
        swp = sb.tile([128, 128], f32, tag="swp")
        nc.sync.dma_start(swp[:], swp_d[:])
        S_m = swp[:]

        nc.vector.memset(cs[:, 0:1], 0.0)

        b16 = sb.tile([128, 1], f32, tag="b16")
        nc.sync.dma_start(b16[:], b16_d[:])

        # ---- build UW table directly in the replicated 128-row layout:
        # U = vm*cos(ang) on u-rows, W = vm*sin on w-rows (bias = pi/2 | 0).
        # ang128/vm128 are host-replicated DRAM inputs; trig reuses the
        # bndG slot (dead until mid-pipeline), ang/vm share one slot.
        trig128 = sb.tile([128, TABW], f32, tag="bndG")
        for c in range(8):
            sl = slice(801 * c, min(801 * (c + 1), TABW))
            w = sl.stop - sl.start
            av_c = sbc.tile([128, 801], f32, tag="bang")
            vm_c = sbc.tile([128, 801], f32, tag="bvm")
            nc.sync.dma_start(av_c[:, :w], ang128_d[:, sl])
            nc.sync.dma_start(vm_c[:, :w], vm128_d[:, sl])
            nc.scalar.activation(trig128[:, sl], av_c[:, :w], AF.Sin, bias=b16[:])
            nc.vector.tensor_mul(tab[:, sl], trig128[:, sl], vm_c[:, :w])


        # ---- boundary-gather output + collective bounce buffers
        bndG = sb.tile([128, 1 + CHUNK], f32, tag="bndG")
        cc_in = dram.tile([128, 1 + CHUNK], f32)
        cc_out = dram.tile([16, 1 + CHUNK], f32)

        # ---- main edge pipeline
        RXC = GC  # rx DMA granularity
        rx_tiles = {}
        bnd1_done = False
        for ga in range(NGA):
            uwg_c = sbc.tile([128, GAC], f32, tag="uwg")
            iasl = slice((GAC // 16) * ga, (GAC // 16) * (ga + 1))
            nc.gpsimd.ap_gather(
                out_ap=uwg_c[:], in_ap=tab[:], idxs_ap=gidx[:, iasl],
                channels=128, num_elems=TABW, d=1, num_idxs=GAC)
            for gci in range(GAC // GC):
                gc = ga * (GAC // GC) + gci
                rx_c = sbX.tile([128, RXC], f32, tag="rx")
                nc.sync.dma_start(rx_c[:], rx_d[:, GC * gc:GC * (gc + 1)])
                rxoff = 0
                usl = slice(GC * gci, GC * (gci + 1))
                t_c = sbX.tile([128, GC], bf16, tag="t")
                s_c = sbX.tile([128, GC], bf16, tag="s")
                sq_c = sbX.tile([128, GC], bf16, tag="sq")
                nc.vector.tensor_mul(t_c[:], uwg_c[:, usl],
                                     rx_c[:, rxoff:rxoff + GC])
                nc.scalar.square(sq_c[:], rx_c[:, rxoff:rxoff + GC])
                for h in range(GC // SUB):
                    ssl = slice(SUB * h, SUB * (h + 1))
                    col0 = GC * gc + SUB * h  # global sub-chunk start
                    psX = ps.tile([128, SUB], f32, tag="psX")
                    psT = ps.tile([128, SUB], f32, tag="psT")
                    psZ = ps.tile([128, SUB], f32, tag="psZ")
                    y_c = sb1.tile([128, SUB], f32, tag="y")
                    # rx swapped across (u,w)-row pairs, via PE permutation
                    nc.tensor.matmul(out=psX[:],
                                     lhsT=S_m,
                                     rhs=rx_c[:, rxoff + SUB * h:
                                              rxoff + SUB * (h + 1)],
                                     start=True, stop=True)
                    nc.vector.tensor_mul(s_c[:, ssl],
                                         uwg_c[:, GC * gci + SUB * h:
                                               GC * gci + SUB * (h + 1)],
                                         psX[:])
                    nc.tensor.matmul(out=psT[:], lhsT=A_m, rhs=t_c[:, ssl],
                                     start=True, stop=False)
                    nc.tensor.matmul(out=psT[:], lhsT=B_m, rhs=s_c[:, ssl],
                                     start=False, stop=True)
                    nc.tensor.matmul(out=psZ[:], lhsT=Gz_m, rhs=sq_c[:, ssl],
                                     start=True, stop=True)
                    nc.scalar.activation(y_c[:], psZ[:], AF.Abs_reciprocal_sqrt)
                    nc.vector._custom_dve(
                        MULSCAN, out=cs[:, 1 + col0:1 + col0 + SUB],
                        in0=psT[:], in1=y_c[:], s0=cs[:, col0:col0 + 1])
                    if col0 + SUB == BND_T1W - 1 and not bnd1_done:
                        # first half of the boundary gather: cs cols
                        # [0, BND_T1W) are final once this sub-chunk lands
                        bnd1_done = True
                        nc.vector.memset(bndG[:, 0:1], 0.0)
                        nc.gpsimd.ap_gather(
                            out_ap=bndG[:, 1:1 + BND_M], in_ap=cs[:, 0:BND_T1W],
                            idxs_ap=bidx[:, 0:BND_M // 16],
                            channels=128, num_elems=BND_T1W, d=1,
                            num_idxs=BND_M)
                        nc.sync.dma_start(cc_in[:, 0:1 + BND_M],
                                          bndG[:, 0:1 + BND_M])

        # ---- tail coefficient tiles (from tab rows 0:16, placed late so
        # their DMAs don't congest the head; tag-share dead build tiles)
        UW128 = sb.tile([128, BLK], f32, tag="ang")
        UW128s = sb.tile([128, BLK], f32, tag="vmb")
        inj = sb.tile([128, BLK], f32, tag="trig")
        vm64 = sb.tile([64, BLK], f32, tag="uwb")
        nc.sync.dma_start(inj[:], inj_d[:])
        for blk in range(8):
            sl = slice(BLK * blk, BLK * (blk + 1))
            rows = slice(16 * blk, 16 * blk + 16)
            nc.scalar.dma_start(UW128[rows, :], tab[0:16, sl])
            nc.scalar.dma_start(UW128s[16 * blk:16 * blk + 8, :], tab[8:16, sl])
            nc.scalar.dma_start(UW128s[16 * blk + 8:16 * blk + 16, :],
                                tab[0:8, sl])
            nc.sync.dma_start(vm64[8 * blk:8 * blk + 8, :], vm128_d[0:8, sl])
        vq_t = sb.tile([64, 1], f32, tag="vqt")
        vscr = sb.tile([128, BLK], bf16, tag="vscr")
        nc.vector._custom_dve(
            VLOSS, out=vscr[0:64, :], in0=vm64[:], s0=V_MIN, s1=V_MAX,
            accum_out=vq_t[:])

        # ---- boundary gather, second half (first half was issued inside
        # the main loop): nodes [BND_M, 6400), table cs[:, BND_OFF2:]
        nc.gpsimd.ap_gather(
            out_ap=bndG[:, 1 + BND_M:1 + CHUNK],
            in_ap=cs[:, BND_OFF2:1 + C_PAD],
            idxs_ap=bidx[:, BND_M // 16:CHUNK // 16],
            channels=128, num_elems=1 + C_PAD - BND_OFF2, d=1,
            num_idxs=CHUNK - BND_M)
        nc.sync.dma_start(cc_in[:, 1 + BND_M:1 + CHUNK],
                          bndG[:, 1 + BND_M:1 + CHUNK])
        if single:
            # timing variant: stand-in DMA instead of the collective
            nc.gpsimd.dma_start(cc_out[:], cc_in[0:16, :])
        else:
            nc.gpsimd.collective_compute(
                "ReduceScatter", mybir.AluOpType.add,
                replica_groups=[list(range(NC))],
                ins=[cc_in.opt()], outs=[cc_out.opt()])

        # ---- tail: this core's 6400 nodes reshaped [16,6401] -> [128,801]
        G128e = sb.tile([128, BLK + 1], f32, tag="uwb")
        for blk in range(8):
            nc.sync.dma_start(G128e[16 * blk:16 * blk + 16, :],
                              cc_out[:, BLK * blk:BLK * (blk + 1) + 1])
        G128 = sb1.tile([128, BLK], f32, tag="y")
        nc.vector.tensor_sub(G128[:], G128e[:, 1:BLK + 1], G128e[:, 0:BLK])
        t128 = sb.tile([128, BLK], bf16, tag="t128")
        s128 = sb.tile([128, BLK], bf16, tag="s128")
        nc.vector.tensor_mul(t128[:], UW128[:], G128[:])
        nc.vector.tensor_mul(s128[:], UW128s[:], G128[:])

        acc0 = sb.tile([128, 1], f32, tag="acc0")
        acc1 = sb.tile([128, 1], f32, tag="acc1")
        acc2 = sb.tile([128, 1], f32, tag="acc2")
        mscr = sb.tile([128, BLK], bf16, tag="vscr")
        nc.vector.memset(acc0[:], 0.0)
        accs = [acc0, acc1, acc2]
        for h in range(2):
            ssl = slice(400 * h, 400 * (h + 1))
            psM = ps.tile([128, 400], f32, tag="psM")
            nc.tensor.matmul(out=psM[:], lhsT=A_m, rhs=t128[:, ssl],
                             start=True, stop=False)
            nc.tensor.matmul(out=psM[:], lhsT=B_m, rhs=s128[:, ssl],
                             start=False, stop=True)
            nc.vector._custom_dve(
                SQDIFF, out=mscr[:, 0:400], in0=psM[:], in1=inj[:, ssl],
                s0=accs[h][:], accum_out=accs[h + 1][:])

        nc.sync.dma_start(misq_d[:], acc2[:])
        nc.sync.dma_start(vq_d[:], vq_t[:])


# --------------------------------------------------------------------------
# module build (cached)
# --------------------------------------------------------------------------

def _build_module(single=False):
    key = "nc1" if single else "nc"
    if key in _CACHE:
        return _CACHE[key], _CACHE["io" + key]
    _register_dve_ops()
    import concourse.bacc as bacc
    import concourse.tile as tile
    from concourse import mybir

    nc = bacc.Bacc("TRN2", target_bir_lowering=False, debug=False,
                   num_devices=1 if single else NC)
    f32, bf16, i16 = mybir.dt.float32, mybir.dt.bfloat16, mybir.dt.int16
    ins = [
        nc.dram_tensor("rx", [128, C_PAD], f32, kind="ExternalInput").ap(),
        nc.dram_tensor("gidx", [128, C_PAD // 16], i16, kind="ExternalInput").ap(),
        nc.dram_tensor("bidx", [128, CHUNK // 16], i16, kind="ExternalInput").ap(),
        nc.dram_tensor("vm128", [128, TABW], f32, kind="ExternalInput").ap(),
        nc.dram_tensor("ang128", [128, TABW], f32, kind="ExternalInput").ap(),
        nc.dram_tensor("inj", [128, BLK], f32, kind="ExternalInput").ap(),
        nc.dram_tensor("mats", [128, 384], bf16, kind="ExternalInput").ap(),
        nc.dram_tensor("swp", [128, 128], f32, kind="ExternalInput").ap(),
        nc.dram_tensor("b16", [128, 1], f32, kind="ExternalInput").ap(),
    ]
    outs = [
        nc.dram_tensor("misq", [128, 1], f32, kind="ExternalOutput").ap(),
        nc.dram_tensor("vq", [64, 1], f32, kind="ExternalOutput").ap(),
    ]
    with tile.TileContext(nc) as tc:
        _trace_kernel(tc, outs, ins, single=single)
    nc.compile()
    _CACHE[key] = nc
    _CACHE["io" + key] = ([t.tensor.name for t in ins],
                          [t.tensor.name for t in outs])
    return nc, _CACHE["io" + key]


# --------------------------------------------------------------------------
# host-side prep / unshard
# --------------------------------------------------------------------------

def _pair_matrices():
    A = np.zeros((128, 128), np.float32)
    Bm = np.zeros((128, 128), np.float32)
    Gz = np.zeros((128, 128), np.float32)
    Sw = np.zeros((128, 128), np.float32)
    for m in range(128):
        g, r = divmod(m, 16)
        if r < 8:  # u-row: re = t_u + t_w
            A[m, m] = 1.0
            A[m + 8, m] = 1.0
            Sw[m + 8, m] = 1.0   # psX[u-row] = rx[w-row] = x
        else:      # w-row: im = s_u - s_w
            Bm[m - 8, m] = 1.0
            Bm[m, m] = -1.0
            Sw[m - 8, m] = 1.0   # psX[w-row] = rx[u-row] = r
        mb = r % 8
        Gz[16 * g + mb, m] = 1.0
        Gz[16 * g + 8 + mb, m] = 1.0
    return A, Bm, Gz, Sw


def _host_prep(v_mag, v_ang, r_line, x_line, p_inj, q_inj, edge_index):
    import ml_dtypes
    frm = np.ascontiguousarray(edge_index[0]).astype(np.int64)
    to = np.ascontiguousarray(edge_index[1]).astype(np.int64)
    core_of = to // CHUNK
    grp_of = frm // CHUNK
    cell = core_of * NG + grp_of
    order = np.lexsort((frm, cell))
    cell_s = cell[order]
    frm_s = frm[order]
    to_s = to[order]
    counts = np.bincount(cell_s, minlength=NC * NG)
    assert counts.max() <= C_PAD, counts.max()
    starts = np.zeros(NC * NG + 1, np.int64)
    np.cumsum(counts, out=starts[1:])
    rank = np.arange(E, dtype=np.int64) - starts[cell_s]

    # padded per-(core,group) slot arrays. Pad edges are distributed so
    # every cell's boundary profile tracks target[n] ~ C_PAD*(n+1)/CHUNK,
    # which keeps the fixed split-table offsets (BND_*) valid for all cells.
    gidx_all = np.full((NC, NG, C_PAD), ZSLOT, np.int32)
    rx_all = np.ones((NC, 128, C_PAD), np.float32)
    bnd_all = np.zeros((NC, NG, CHUNK), np.int64)
    kk, gg = core_of[order], grp_of[order]
    target = np.ceil(C_PAD * (np.arange(CHUNK) + 1.0) / CHUNK).astype(np.int64)
    slot_of = np.empty(E, np.int64)
    for k in range(NC):
        for g in range(NG):
            c0, c1 = starts[k * NG + g], starts[k * NG + g + 1]
            cnt = c1 - c0
            nloc = frm_s[c0:c1] - CHUNK * g         # [cnt] node-local, sorted
            cum_real = np.searchsorted(nloc, np.arange(CHUNK), side="right")
            pads_used = np.maximum.accumulate(np.maximum(target - cum_real, 0))
            pads_used = np.minimum(pads_used, C_PAD - cnt)
            bnd = cum_real + pads_used
            bnd_all[k, g] = bnd
            # real edge i of node n -> slot i + pads_used[n-1]
            pu_prev = np.concatenate([[0], pads_used[:-1]])
            slot_of[c0:c1] = np.arange(cnt) + pu_prev[nloc]
    sidx = slot_of
    gidx_all[kk, gg, sidx] = (to_s - CHUNK * kk).astype(np.int32)
    eidx = order  # original edge ids in sorted order
    for b in range(B):
        rx_all[kk, 16 * gg + b, sidx] = r_line[b][eidx]
        rx_all[kk, 16 * gg + 8 + b, sidx] = x_line[b][eidx]

    A, Bm, Gz, Sw = _pair_matrices()
    bias16 = np.tile(np.array([[np.pi / 2]] * 8 + [[0.0]] * 8, np.float32), (8, 1))
    mats = np.concatenate([A, Bm, Gz], axis=1).astype(ml_dtypes.bfloat16)

    in_maps = []
    for k in range(NC):
        # wrapped gather idx (per GAC-call): row 16g+p, col (GAC//16)*c + s
        #   holds idx[g, GAC*c + 16*s + p]
        gi = gidx_all[k].reshape(NG, NGA, GAC // 16, 16)
        gi = gi.transpose(0, 3, 1, 2).reshape(NG * 16, NGA * (GAC // 16))
        gidx_w = gi.astype(np.int16)

        # boundary positions, split into two calls (see BND_* constants)
        bidx_w = np.zeros((128, CHUNK // 16), np.int16)
        lo = CHUNK * k
        hi = min(CHUNK * (k + 1), N)
        n_real = hi - lo
        for g in range(NG):
            pos = bnd_all[k, g]
            p1 = pos[:BND_M]
            p2 = pos[BND_M:] - BND_OFF2
            assert p1.max() < BND_T1W, (k, g, p1.max())
            assert p2.min() >= 0, (k, g, p2.min())
            bidx_w[16 * g:16 * g + 16, :BND_M // 16] = (
                p1.reshape(BND_M // 16, 16).T.astype(np.int16))
            bidx_w[16 * g:16 * g + 16, BND_M // 16:] = (
                p2.reshape((CHUNK - BND_M) // 16, 16).T.astype(np.int16))

        vm16 = np.zeros((16, TABW), np.float32)
        ang16 = np.zeros((16, TABW), np.float32)
        for uw in range(2):
            vm16[uw * 8:uw * 8 + 8, :n_real] = v_mag[:, lo:hi]
            vm16[uw * 8:uw * 8 + 8, n_real:CHUNK] = 1.0
            ang16[uw * 8:uw * 8 + 8, :n_real] = v_ang[:, lo:hi]
        vm128 = np.tile(vm16, (8, 1))
        ang128 = np.tile(ang16, (8, 1))

        inj = np.zeros((128, BLK), np.float32)
        for blk in range(8):
            nlo = lo + BLK * blk
            w = max(0, min(nlo + BLK, hi) - nlo)
            if w > 0:
                inj[blk * 16:blk * 16 + 8, :w] = p_inj[:, nlo:nlo + w]
                inj[blk * 16 + 8:blk * 16 + 16, :w] = q_inj[:, nlo:nlo + w]

        in_maps.append({
            "rx": rx_all[k],
            "gidx": gidx_w,
            "bidx": bidx_w,
            "vm128": vm128,
            "ang128": ang128,
            "inj": inj,
            "mats": mats,
            "swp": Sw,
            "b16": bias16,
        })
    return in_maps


def kernel(**inputs):
    inputs = {k: np.asarray(v) for k, v in inputs.items()}
    nc, _ = _build_module()
    in_maps = _host_prep(**inputs)
    from concourse.bass_utils import run_bass_kernel_spmd
    res = run_bass_kernel_spmd(nc, in_maps, core_ids=list(range(NC)))
    mis_total = 0.0
    v_total = 0.0
    for k in range(NC):
        mis_total += float(res.results[k]["misq"].sum(dtype=np.float64))
        v_total += float(res.results[k]["vq"].sum(dtype=np.float64))
    loss = np.float32(W_PB * (mis_total / B) + v_total / B)
    return inputs["v_mag"], inputs["v_ang"], loss
